# revision 47
# baseline (speedup 1.0000x reference)
"""Causal single-head attention (B=4, S=2048, d=1024) on 8 TRN2 NeuronCores.

Sharding (uniform single program): core c -> batch b = c//2, subset
s = c%2. Per batch, the 16 query blocks of 128 rows are split between
the pair in a causally BALANCED way: core (b,s) owns the low blocks
{2g+s : g=0..3} and the high blocks {15-2g-s : g=0..3}, so both cores
carry identical padded causal work (72 key-block-pairs each vs 80 for
a contiguous split). Every core runs the identical instruction stream;
the true causal boundary comes from per-core 0/1 mask tiles supplied
as input data (one [128,128] mask per key block).

K/V projections are tensor-parallel within each core pair: core (b,s)
computes the d_out-half s of kT and v for the whole batch; halves are
exchanged with one 2MB pairwise AllGather each
([[0,1],[2,3],[4,5],[6,7]]); a tiny warmup collective absorbs the
one-time stream barrier + start delay while projections run.

Schedule: K projection streams over resident 512-key x chunks (loaded
once via two parallel DMA queues), K AllGather, V projection reusing
the resident chunks, V AllGather, Q projection. Attention: scores for
the LOW query group (kb<8: only the first K gather group), then the
HIGH group, then row sums + AV per owned block in descending chain
length so the kernel tail is the shortest chain.

Compute (bf16 operands, fp32 PSUM accumulate):
  scoresT[k,q] = kt.T-slices @ qT-group, p = exp(scoresT)
  (no max subtraction: |scores| <= ~2), mask, then
  out[q,:] = (pT.T @ v) / (pT.T @ ones)  -- row sums via ones-matmul.
"""
import sys

sys.path.insert(0, "/opt/trn_rl_repo")

import ml_dtypes
import numpy as np

import concourse.bass as bass  # noqa: F401
import concourse.mybir as mybir
import concourse.tile as tile
from concourse import bacc
from concourse.bass_utils import run_bass_kernel_spmd

B, S, D = 4, 2048, 1024
DC = D // 128          # 8 contraction chunks
NKB = S // 128         # 16 key blocks
SCALE = 1.0 / float(np.sqrt(D))
F32 = mybir.dt.float32
BF = mybir.dt.bfloat16
EXP = mybir.ActivationFunctionType.Exp
GROUPS = [[0, 1], [2, 3], [4, 5], [6, 7]]

_cache = {}


def _hi_nb(kb):
    """Number of HIGH-group 128-col blocks that attend key block kb."""
    return min(4, (15 - kb) // 2 + 1)


def build_nc():
    nc = bacc.Bacc("TRN2", target_bir_lowering=False, debug=False, num_devices=8)
    # all inputs partition-major: [128, dc, cols]
    xT = nc.dram_tensor("xT", [128, DC, S], BF, kind="ExternalInput")
    xTq = nc.dram_tensor("xTq", [128, DC, 1024], BF, kind="ExternalInput")
    WqT = nc.dram_tensor("WqT", [128, DC, D], BF, kind="ExternalInput")
    WkTh = nc.dram_tensor("WkTh", [128, DC, 512], BF, kind="ExternalInput")
    WvTh = nc.dram_tensor("WvTh", [128, DC, 512], BF, kind="ExternalInput")
    masks = nc.dram_tensor("masks", [128, 16, 128], BF, kind="ExternalInput")
    out = nc.dram_tensor("out", [1024, D], F32, kind="ExternalOutput")
    kg_in = nc.dram_tensor("kg_in", [2, 128, 4, 1024], BF)
    kg_out = nc.dram_tensor("kg_out", [2, 2, 128, 4, 1024], BF)  # [g][rank]
    vg_in = nc.dram_tensor("vg_in", [2, 128, 8, 512], BF)
    vg_out = nc.dram_tensor("vg_out", [2, 2, 128, 8, 512], BF)
    warm_in = nc.dram_tensor("warm_in", [1, 64], BF)
    warm_out = nc.dram_tensor("warm_out", [2, 64], BF)

    with tile.TileContext(nc) as tc:
        with (
            tc.tile_pool(name="w", bufs=1) as wp,
            tc.tile_pool(name="per", bufs=1) as per,
            tc.tile_pool(name="px", bufs=4) as pxp,
            tc.tile_pool(name="stg", bufs=1) as stg,
            tc.tile_pool(name="p512", bufs=12) as pp512,
            tc.tile_pool(name="p384", bufs=4) as pp384,
            tc.tile_pool(name="p256", bufs=4) as pp256,
            tc.tile_pool(name="p128", bufs=4) as pp128,
            tc.tile_pool(name="ot", bufs=4) as otp,
            tc.tile_pool(name="sml", bufs=4) as smlp,
            tc.tile_pool(name="mix", bufs=5, space="PSUM") as mixp,
            tc.tile_pool(name="psav", bufs=3, space="PSUM") as psavp,
        ):
            # ---------------- consts + persistent ----------------
            # kt/vv split per 1024-key gather group for fine-grained readiness
            kts = [per.tile([128, DC, 1024], BF, name=f"kt{g}") for g in range(2)]
            vvs = [per.tile([128, 2, 8, 512], BF, name=f"vv{g}") for g in range(2)]
            qt = per.tile([128, DC, 1024], BF)     # qT: [d_out, 1024]
            zeros_f = per.tile([128, 2], F32)
            ones = per.tile([128, 2], BF)
            maskt = per.tile([128, 16, 128], BF)
            nc.vector.memset(zeros_f, 0.0)
            # exp(0)=1 -> also preloads the ACT exp table long before attention
            nc.scalar.activation(ones, zeros_f, EXP)

            wk = wp.tile([128, DC, 512], BF)
            wv = wp.tile([128, DC, 512], BF)
            wq = wp.tile([128, DC, D], BF)
            xq = wp.tile([128, DC, 1024], BF)
            # startup: wk halves on sync while xk0 halves arrive via the
            # scalar queue (two parallel queues); the first matmul chain
            # starts once the dc 0-3 halves of both land.
            nc.sync.dma_start(out=wk[:, 0:4, :], in_=WkTh[:, 0:4, :])
            nc.sync.dma_start(out=wk[:, 4:8, :], in_=WkTh[:, 4:8, :])
            xks = []
            for sc in range(4):
                xk = pxp.tile([128, DC, 512], BF, tag="xs", name=f"xk_{sc}")
                if sc == 0:
                    nc.scalar.dma_start(out=xk[:, 0:4, :], in_=xT[:, 0:4, 0:512])
                    nc.scalar.dma_start(out=xk[:, 4:8, :], in_=xT[:, 4:8, 0:512])
                else:
                    nc.sync.dma_start(
                        out=xk, in_=xT[:, :, sc * 512:(sc + 1) * 512]
                    )
                xks.append(xk)
            # Q inputs: finish before the gathers occupy the fabric
            nc.sync.dma_start(out=wq, in_=WqT[:])
            nc.sync.dma_start(out=xq, in_=xTq[:])
            nc.scalar.dma_start(out=wv, in_=WvTh[:])
            nc.scalar.dma_start(out=maskt, in_=masks[:])

            # -------- K half-projection -> one 2MB AllGather --------
            # single g-major staging tile: half g stores as soon as its
            # casts land, with no pool-rotation serialization
            kgs = stg.tile([128, 2, 4, 1024], BF, tag="kgs", bufs=1)
            for g in range(2):
                for scl in range(2):
                    xk = xks[2 * g + scl]
                    for ocl in range(4):
                        ps = mixp.tile([128, 512], F32, tag="mix")
                        for dc in range(DC):
                            nc.tensor.matmul(
                                ps,
                                lhsT=wk[:, dc, ocl * 128:(ocl + 1) * 128],
                                rhs=xk[:, dc, :],
                                start=(dc == 0),
                                stop=(dc == DC - 1),
                            )
                        nc.vector.tensor_copy(
                            kgs[:, g, ocl, scl * 512:(scl + 1) * 512], ps
                        )
                nc.scalar.dma_start(out=kg_in[g], in_=kgs[:, g])
                # per-group 1MB gathers: group 0's input is stored (~39us)
                # before the one-time stream barrier ends, so the first op
                # absorbs the stream start delay itself and kt group 0 lands
                # ~15us before the scores need it
                nc.gpsimd.collective_compute(
                    "AllGather",
                    mybir.AluOpType.bypass,
                    replica_groups=GROUPS,
                    ins=[kg_in[g]],
                    outs=[kg_out[g]],
                )

            # -------- V half-projection (resident x) -> one 2MB AllGather ---
            # reuses the kgs staging slot (same 16KB footprint; the K stores
            # drain just as the first V casts arrive)
            vgs = stg.tile([128, 2, 8, 512], BF, tag="kgs", bufs=1)
            for g in range(2):
                for scl in range(2):
                    xk = xks[2 * g + scl]
                    for sb in range(4):
                        ps = mixp.tile([128, 512], F32, tag="mix",
                                       name=f"ps2_{g}_{scl}_{sb}")
                        for dc in range(DC):
                            nc.tensor.matmul(
                                ps,
                                lhsT=xk[:, dc, sb * 128:(sb + 1) * 128],
                                rhs=wv[:, dc, :],
                                start=(dc == 0),
                                stop=(dc == DC - 1),
                            )
                        nc.vector.tensor_copy(vgs[:, g, scl * 4 + sb, :], ps)
                if g == 0:
                    # kt group 0 loads jump the scalar queue here: they beat
                    # the V gathers onto the fabric, so the scores never
                    # wait; the V trigger slides back only ~4us, covered by
                    # the LOW-first AV ordering.
                    nc.scalar.dma_start(
                        out=kts[0][:, 4:8, :], in_=kg_out[0, 1]
                    )
                nc.scalar.dma_start(out=vg_in[g], in_=vgs[:, g])
                # V gathers as two 1MB ops: group 0 lands ~20us earlier,
                # feeding the first AV chains while group 1 still transfers
                nc.gpsimd.collective_compute(
                    "AllGather",
                    mybir.AluOpType.bypass,
                    replica_groups=GROUPS,
                    ins=[vg_in[g]],
                    outs=[vg_out[g]],
                )

            # gathered K -> SBUF: each group's two ranks split across the
            # sync and scalar queues so group 0 lands in half the time
            # (scalar's block ends at K-gather completion, well before the
            # attention exps it must later feed). V -> SBUF on sync.
            for g in range(2):
                nc.sync.dma_start(out=kts[g][:, 0:4, :], in_=kg_out[g, 0])
            nc.scalar.dma_start(out=kts[1][:, 4:8, :], in_=kg_out[1, 1])
            for g in range(2):
                for r in range(2):
                    nc.sync.dma_start(
                        out=vvs[g][:, r, :, :], in_=vg_out[g, r]
                    )

            # -------- Q projection -> qt --------
            for sc in range(2):
                for oc in range(8):
                    ps = mixp.tile([128, 512], F32, tag="mix",
                                   name=f"ps0_{sc}_{oc}")
                    for dc in range(DC):
                        nc.tensor.matmul(
                            ps,
                            lhsT=wq[:, dc, oc * 128:(oc + 1) * 128],
                            rhs=xq[:, dc, sc * 512:(sc + 1) * 512],
                            start=(dc == 0),
                            stop=(dc == DC - 1),
                        )
                    nc.vector.tensor_copy(
                        qt[:, oc, sc * 512:(sc + 1) * 512], ps
                    )

            # ---------------- attention ----------------
            pools = {512: pp512, 384: pp384, 256: pp256, 128: pp128}
            pt_lo, pt_hi = {}, {}

            def kslice(kb, dc):
                return kts[kb // 8][:, dc, (kb % 8) * 128:(kb % 8 + 1) * 128]

            def emit_scores_lo():
                # LOW group: owned block 2g+s lives at qt cols [128g, 128g+128);
                # key block kb is attended by col-blocks g >= kb//2.
                for kb in range(8):
                    j = kb // 2
                    W = 512 - 128 * j
                    ps = mixp.tile([128, 512], F32, tag="mix",
                                   name=f"slo_{kb}")
                    for dc in range(DC):
                        nc.tensor.matmul(
                            ps[:, 0:W],
                            lhsT=kslice(kb, dc),
                            rhs=qt[:, dc, 128 * j:512],
                            start=(dc == 0),
                            stop=(dc == DC - 1),
                        )
                    pt = pools[W].tile([128, W], BF, tag=f"p{W}",
                                       name=f"plo_{kb}")
                    nc.scalar.activation(pt, ps[:, 0:W], EXP)
                    # the boundary block is always this tile's first 128 cols
                    nc.vector.tensor_mul(
                        pt[:, 0:128], pt[:, 0:128], maskt[:, kb, :]
                    )
                    pt_lo[kb] = pt

            def emit_scores_hi():
                # HIGH group: owned block 15-2g-s lives at qt cols
                # [512+128g, ...); key block kb is attended by col-blocks
                # g <= (15-kb)//2 (all four for kb < 8).
                for kb in range(16):
                    W = 128 * _hi_nb(kb)
                    ps = mixp.tile([128, 512], F32, tag="mix",
                                   name=f"shi_{kb}")
                    for dc in range(DC):
                        nc.tensor.matmul(
                            ps[:, 0:W],
                            lhsT=kslice(kb, dc),
                            rhs=qt[:, dc, 512:512 + W],
                            start=(dc == 0),
                            stop=(dc == DC - 1),
                        )
                    pt = pools[W].tile([128, W], BF, tag=f"p{W}",
                                       name=f"phi_{kb}")
                    nc.scalar.activation(pt, ps[:, 0:W], EXP)
                    if kb >= 8:
                        # the boundary block is this tile's last 128 cols
                        nc.vector.tensor_mul(
                            pt[:, W - 128:W], pt[:, W - 128:W], maskt[:, kb, :]
                        )
                    pt_hi[kb] = pt

            def emit_av(sec, g):
                # one owned 128-row block: row sums, then AV per d_out half
                if sec == "lo":
                    L = 2 * g + 2
                    row = 128 * g

                    def slc(kb):
                        return pt_lo[kb][:, 128 * (g - kb // 2):
                                         128 * (g - kb // 2) + 128]
                else:
                    L = 16 - 2 * g
                    row = 512 + 128 * g

                    def slc(kb):
                        return pt_hi[kb][:, 128 * g:128 * (g + 1)]

                lps = psavp.tile([128, 2], F32, tag="psav",
                                 name=f"l_{sec}_{g}")
                for kb in range(L):
                    nc.tensor.matmul(
                        lps,
                        lhsT=slc(kb),
                        rhs=ones,
                        start=(kb == 0),
                        stop=(kb == L - 1),
                    )
                rec = smlp.tile([128, 1], F32, tag="rec")
                nc.vector.reciprocal(rec, lps[:, 0:1])
                for oh in range(2):
                    avp = psavp.tile([128, 512], F32, tag="psav",
                                     name=f"av_{sec}_{g}_{oh}")
                    for kb in range(L):
                        nc.tensor.matmul(
                            avp,
                            lhsT=slc(kb),
                            rhs=vvs[kb // 8][:, oh, kb % 8, :],
                            start=(kb == 0),
                            stop=(kb == L - 1),
                        )
                    ot = otp.tile([128, 512], F32, tag="ot",
                                  name=f"ot_{sec}_{g}_{oh}")
                    nc.vector.tensor_scalar_mul(ot, avp, rec)
                    # split output stores across two queues: the tail
                    # otherwise serializes on a single queue's drain
                    eng = nc.sync if sec == "hi" else nc.scalar
                    eng.dma_start(
                        out=out[row:row + 128, oh * 512:(oh + 1) * 512],
                        in_=ot,
                    )

            # LOW scores first: they only need the first K gather group.
            # AV: LOW blocks first -- they only read the first V gather
            # group, buying the trailing V gather ~15us of slack.
            emit_scores_lo()
            emit_scores_hi()
            for g in (3, 2, 1):
                emit_av("lo", g)
            for g in range(4):
                emit_av("hi", g)
            # the L=2 chain last: shortest possible kernel tail
            emit_av("lo", 0)
    nc.compile()
    return nc


def _query_blocks(sub):
    return [2 * g + sub for g in range(4)] + [15 - 2 * g - sub for g in range(4)]


def _query_cols(sub):
    return np.concatenate(
        [np.arange(b * 128, (b + 1) * 128) for b in _query_blocks(sub)]
    )


def _masks(sub):
    m = np.zeros((16, 128, 128), np.float32)
    k = np.arange(128)[:, None]
    q = np.arange(128)[None, :]
    for kb in range(16):
        if kb < 8:
            block = 2 * (kb // 2) + sub
        else:
            block = 15 - 2 * ((15 - kb) // 2) - sub
        m[kb] = (kb * 128 + k <= block * 128 + q).astype(np.float32)
    return np.ascontiguousarray(m.transpose(1, 0, 2))  # -> [128, 16, 128]


def _pmaj(a):
    """[dc*128, cols] -> partition-major [128, dc, cols]."""
    d, cols = a.shape
    return np.ascontiguousarray(a.reshape(d // 128, 128, cols).transpose(1, 0, 2))


def kernel(x, Wq, Wk, Wv, _trace=False):
    if "nc" not in _cache:
        _cache["nc"] = build_nc()
    nc = _cache["nc"]

    bf = ml_dtypes.bfloat16
    x = np.asarray(x, dtype=np.float32)
    WqT = _pmaj((np.asarray(Wq, np.float32).T * np.float32(SCALE)).astype(bf))
    WkT = np.asarray(Wk, np.float32).T.astype(bf)
    WvT = np.asarray(Wv, np.float32).T.astype(bf)

    in_maps = []
    for c in range(8):
        b, sub = c // 2, c % 2
        xT = x[b].T.astype(bf)
        in_maps.append(
            {
                "xT": _pmaj(xT),
                "xTq": _pmaj(np.ascontiguousarray(xT[:, _query_cols(sub)])),
                "WqT": WqT,
                "WkTh": _pmaj(WkT[:, sub * 512:(sub + 1) * 512]),
                "WvTh": _pmaj(WvT[:, sub * 512:(sub + 1) * 512]),
                "masks": _masks(sub).astype(bf),
            }
        )

    res = run_bass_kernel_spmd(
        nc, in_maps, core_ids=list(range(8)), trace=_trace
    )
    full = np.empty((B, S, D), np.float32)
    for c in range(8):
        b, sub = c // 2, c % 2
        full[b, _query_cols(sub)] = res.results[c]["out"]
    if _trace:
        _cache["last_result"] = res
    return full


# revision 51
# speedup vs baseline: 1.0850x; 1.0850x over previous
"""Causal single-head attention (B=4, S=2048, d=1024) on 8 TRN2 NeuronCores.

Sharding (uniform single program): core c -> batch b = c//2, subset
s = c%2. Per batch, the 16 query blocks of 128 rows are split between
the pair in a causally BALANCED way: core (b,s) owns the low blocks
{2g+s : g=0..3} and the high blocks {15-2g-s : g=0..3}, so both cores
carry identical padded causal work (72 key-block-pairs each vs 80 for
a contiguous split). Every core runs the identical instruction stream;
the true causal boundary comes from per-core 0/1 mask tiles supplied
as input data (one [128,128] mask per key block).

K/V projections are tensor-parallel within each core pair: core (b,s)
computes the d_out-half s of kT and v for the whole batch; halves are
exchanged with one 2MB pairwise AllGather each
([[0,1],[2,3],[4,5],[6,7]]); a tiny warmup collective absorbs the
one-time stream barrier + start delay while projections run.

Schedule: K projection streams over resident 512-key x chunks (loaded
once via two parallel DMA queues), K AllGather, V projection reusing
the resident chunks, V AllGather, Q projection. Attention: scores for
the LOW query group (kb<8: only the first K gather group), then the
HIGH group, then row sums + AV per owned block in descending chain
length so the kernel tail is the shortest chain.

Compute (bf16 operands, fp32 PSUM accumulate):
  scoresT[k,q] = kt.T-slices @ qT-group, p = exp(scoresT)
  (no max subtraction: |scores| <= ~2), mask, then
  out[q,:] = (pT.T @ v) / (pT.T @ ones)  -- row sums via ones-matmul.
"""
import sys

sys.path.insert(0, "/opt/trn_rl_repo")

import ml_dtypes
import numpy as np

import concourse.bass as bass  # noqa: F401
import concourse.mybir as mybir
import concourse.tile as tile
from concourse import bacc
from concourse.bass_utils import run_bass_kernel_spmd

B, S, D = 4, 2048, 1024
DC = D // 128          # 8 contraction chunks
NKB = S // 128         # 16 key blocks
SCALE = 1.0 / float(np.sqrt(D))
F32 = mybir.dt.float32
BF = mybir.dt.bfloat16
EXP = mybir.ActivationFunctionType.Exp
GROUPS = [[0, 1], [2, 3], [4, 5], [6, 7]]

_cache = {}


def _hi_nb(kb):
    """Number of HIGH-group 128-col blocks that attend key block kb."""
    return min(4, (15 - kb) // 2 + 1)


def build_nc():
    nc = bacc.Bacc("TRN2", target_bir_lowering=False, debug=False, num_devices=8)
    # all inputs partition-major: [128, dc, cols]
    xT = nc.dram_tensor("xT", [128, DC, S], BF, kind="ExternalInput")
    xTq = nc.dram_tensor("xTq", [128, DC, 1024], BF, kind="ExternalInput")
    WqT = nc.dram_tensor("WqT", [128, DC, D], BF, kind="ExternalInput")
    WkTh = nc.dram_tensor("WkTh", [128, DC, 512], BF, kind="ExternalInput")
    WvTh = nc.dram_tensor("WvTh", [128, DC, 512], BF, kind="ExternalInput")
    masks = nc.dram_tensor("masks", [128, 16, 128], BF, kind="ExternalInput")
    out = nc.dram_tensor("out", [1024, D], F32, kind="ExternalOutput")
    kg_in = nc.dram_tensor("kg_in", [2, 128, 4, 1024], BF)
    kg_out = nc.dram_tensor("kg_out", [2, 2, 128, 4, 1024], BF)  # [g][rank]
    vg_in = nc.dram_tensor("vg_in", [2, 128, 8, 512], BF)
    vg_out = nc.dram_tensor("vg_out", [2, 2, 128, 8, 512], BF)
    warm_in = nc.dram_tensor("warm_in", [1, 64], BF)
    warm_out = nc.dram_tensor("warm_out", [2, 64], BF)

    with tile.TileContext(nc) as tc:
        with (
            tc.tile_pool(name="w", bufs=1) as wp,
            tc.tile_pool(name="per", bufs=1) as per,
            tc.tile_pool(name="px", bufs=4) as pxp,
            tc.tile_pool(name="stg", bufs=1) as stg,
            tc.tile_pool(name="p512", bufs=12) as pp512,
            tc.tile_pool(name="p384", bufs=4) as pp384,
            tc.tile_pool(name="p256", bufs=4) as pp256,
            tc.tile_pool(name="p128", bufs=4) as pp128,
            tc.tile_pool(name="ot", bufs=4) as otp,
            tc.tile_pool(name="sml", bufs=4) as smlp,
            tc.tile_pool(name="mix", bufs=5, space="PSUM") as mixp,
            tc.tile_pool(name="psav", bufs=3, space="PSUM") as psavp,
        ):
            # tiny warmup AllGather (garbage data, nobody reads it): absorbs
            # the one-time collective-stream barrier + start delay while the
            # projections still run. gpsimd carries no loads, so this cannot
            # block any data path.
            nc.gpsimd.collective_compute(
                "AllGather",
                mybir.AluOpType.bypass,
                replica_groups=GROUPS,
                ins=[warm_in[:]],
                outs=[warm_out[:]],
            )

            # ---------------- consts + persistent ----------------
            # kt/vv split per 1024-key gather group for fine-grained readiness
            kts = [per.tile([128, DC, 1024], BF, name=f"kt{g}") for g in range(2)]
            vvs = [per.tile([128, 2, 8, 512], BF, name=f"vv{g}") for g in range(2)]
            qt = per.tile([128, DC, 1024], BF)     # qT: [d_out, 1024]
            zeros_f = per.tile([128, 2], F32)
            ones = per.tile([128, 2], BF)
            maskt = per.tile([128, 16, 128], BF)
            nc.vector.memset(zeros_f, 0.0)
            # exp(0)=1 -> also preloads the ACT exp table long before attention
            nc.scalar.activation(ones, zeros_f, EXP)

            wk = wp.tile([128, DC, 512], BF)
            wv = wp.tile([128, DC, 512], BF)
            wq = wp.tile([128, DC, D], BF)
            xq = wp.tile([128, DC, 1024], BF)
            # startup: wk halves on sync while xk0 halves arrive via the
            # scalar queue (two parallel queues); the first matmul chain
            # starts once the dc 0-3 halves of both land.
            nc.sync.dma_start(out=wk[:, 0:4, :], in_=WkTh[:, 0:4, :])
            nc.sync.dma_start(out=wk[:, 4:8, :], in_=WkTh[:, 4:8, :])
            xks = []
            for sc in range(4):
                xk = pxp.tile([128, DC, 512], BF, tag="xs", name=f"xk_{sc}")
                if sc == 0:
                    nc.scalar.dma_start(out=xk[:, 0:4, :], in_=xT[:, 0:4, 0:512])
                    nc.scalar.dma_start(out=xk[:, 4:8, :], in_=xT[:, 4:8, 0:512])
                else:
                    nc.sync.dma_start(
                        out=xk, in_=xT[:, :, sc * 512:(sc + 1) * 512]
                    )
                xks.append(xk)
            # Q inputs: finish before the gathers occupy the fabric
            nc.sync.dma_start(out=wq, in_=WqT[:])
            nc.sync.dma_start(out=xq, in_=xTq[:])
            nc.scalar.dma_start(out=wv, in_=WvTh[:])
            nc.scalar.dma_start(out=maskt, in_=masks[:])

            # -------- K half-projection -> one 2MB AllGather --------
            # single g-major staging tile: half g stores as soon as its
            # casts land, with no pool-rotation serialization
            kgs = stg.tile([128, 2, 4, 1024], BF, tag="kgs", bufs=1)
            for g in range(2):
                for scl in range(2):
                    xk = xks[2 * g + scl]
                    for ocl in range(4):
                        ps = mixp.tile([128, 512], F32, tag="mix")
                        for dc in range(DC):
                            nc.tensor.matmul(
                                ps,
                                lhsT=wk[:, dc, ocl * 128:(ocl + 1) * 128],
                                rhs=xk[:, dc, :],
                                start=(dc == 0),
                                stop=(dc == DC - 1),
                            )
                        nc.vector.tensor_copy(
                            kgs[:, g, ocl, scl * 512:(scl + 1) * 512], ps
                        )
                nc.scalar.dma_start(out=kg_in[g], in_=kgs[:, g])
            # one 2MB K gather: per-op stream overhead (~9us) makes two
            # 1MB ops slower end-to-end than one 2MB op
            nc.gpsimd.collective_compute(
                "AllGather",
                mybir.AluOpType.bypass,
                replica_groups=GROUPS,
                ins=[kg_in[:]],
                outs=[kg_out[:]],
            )

            # -------- V half-projection (resident x) -> one 2MB AllGather ---
            # reuses the kgs staging slot (same 16KB footprint; the K stores
            # drain just as the first V casts arrive)
            vgs = stg.tile([128, 2, 8, 512], BF, tag="kgs", bufs=1)
            for g in range(2):
                for scl in range(2):
                    xk = xks[2 * g + scl]
                    for sb in range(4):
                        ps = mixp.tile([128, 512], F32, tag="mix",
                                       name=f"ps2_{g}_{scl}_{sb}")
                        for dc in range(DC):
                            nc.tensor.matmul(
                                ps,
                                lhsT=xk[:, dc, sb * 128:(sb + 1) * 128],
                                rhs=wv[:, dc, :],
                                start=(dc == 0),
                                stop=(dc == DC - 1),
                            )
                        nc.vector.tensor_copy(vgs[:, g, scl * 4 + sb, :], ps)
                if g == 0:
                    # kt group 0 loads jump the scalar queue here: they beat
                    # the V gathers onto the fabric, so the scores never
                    # wait; the V trigger slides back only ~4us, covered by
                    # the LOW-first AV ordering.
                    nc.scalar.dma_start(
                        out=kts[0][:, 4:8, :], in_=kg_out[1, 0]
                    )
                nc.scalar.dma_start(out=vg_in[g], in_=vgs[:, g])
                # V gathers as two 1MB ops: group 0 lands ~20us earlier,
                # feeding the first AV chains while group 1 still transfers
                nc.gpsimd.collective_compute(
                    "AllGather",
                    mybir.AluOpType.bypass,
                    replica_groups=GROUPS,
                    ins=[vg_in[g]],
                    outs=[vg_out[g]],
                )

            # gathered K -> SBUF: each group's two ranks split across the
            # sync and scalar queues so group 0 lands in half the time
            # (scalar's block ends at K-gather completion, well before the
            # attention exps it must later feed). V -> SBUF on sync.
            for g in range(2):
                nc.sync.dma_start(out=kts[g][:, 0:4, :], in_=kg_out[0, g])
            nc.scalar.dma_start(out=kts[1][:, 4:8, :], in_=kg_out[1, 1])
            for g in range(2):
                for r in range(2):
                    nc.sync.dma_start(
                        out=vvs[g][:, r, :, :], in_=vg_out[g, r]
                    )

            # -------- Q projection -> qt --------
            for sc in range(2):
                for oc in range(8):
                    ps = mixp.tile([128, 512], F32, tag="mix",
                                   name=f"ps0_{sc}_{oc}")
                    for dc in range(DC):
                        nc.tensor.matmul(
                            ps,
                            lhsT=wq[:, dc, oc * 128:(oc + 1) * 128],
                            rhs=xq[:, dc, sc * 512:(sc + 1) * 512],
                            start=(dc == 0),
                            stop=(dc == DC - 1),
                        )
                    nc.vector.tensor_copy(
                        qt[:, oc, sc * 512:(sc + 1) * 512], ps
                    )

            # ---------------- attention ----------------
            pools = {512: pp512, 384: pp384, 256: pp256, 128: pp128}
            pt_lo, pt_hi = {}, {}

            def kslice(kb, dc):
                return kts[kb // 8][:, dc, (kb % 8) * 128:(kb % 8 + 1) * 128]

            def emit_scores_lo():
                # LOW group: owned block 2g+s lives at qt cols [128g, 128g+128);
                # key block kb is attended by col-blocks g >= kb//2.
                for kb in range(8):
                    j = kb // 2
                    W = 512 - 128 * j
                    ps = mixp.tile([128, 512], F32, tag="mix",
                                   name=f"slo_{kb}")
                    for dc in range(DC):
                        nc.tensor.matmul(
                            ps[:, 0:W],
                            lhsT=kslice(kb, dc),
                            rhs=qt[:, dc, 128 * j:512],
                            start=(dc == 0),
                            stop=(dc == DC - 1),
                        )
                    pt = pools[W].tile([128, W], BF, tag=f"p{W}",
                                       name=f"plo_{kb}")
                    nc.scalar.activation(pt, ps[:, 0:W], EXP)
                    # the boundary block is always this tile's first 128 cols
                    nc.vector.tensor_mul(
                        pt[:, 0:128], pt[:, 0:128], maskt[:, kb, :]
                    )
                    pt_lo[kb] = pt

            def emit_scores_hi():
                # HIGH group: owned block 15-2g-s lives at qt cols
                # [512+128g, ...); key block kb is attended by col-blocks
                # g <= (15-kb)//2 (all four for kb < 8).
                for kb in range(16):
                    W = 128 * _hi_nb(kb)
                    ps = mixp.tile([128, 512], F32, tag="mix",
                                   name=f"shi_{kb}")
                    for dc in range(DC):
                        nc.tensor.matmul(
                            ps[:, 0:W],
                            lhsT=kslice(kb, dc),
                            rhs=qt[:, dc, 512:512 + W],
                            start=(dc == 0),
                            stop=(dc == DC - 1),
                        )
                    pt = pools[W].tile([128, W], BF, tag=f"p{W}",
                                       name=f"phi_{kb}")
                    nc.scalar.activation(pt, ps[:, 0:W], EXP)
                    if kb >= 8:
                        # the boundary block is this tile's last 128 cols
                        nc.vector.tensor_mul(
                            pt[:, W - 128:W], pt[:, W - 128:W], maskt[:, kb, :]
                        )
                    pt_hi[kb] = pt

            def emit_av(sec, g):
                # one owned 128-row block: row sums, then AV per d_out half
                if sec == "lo":
                    L = 2 * g + 2
                    row = 128 * g

                    def slc(kb):
                        return pt_lo[kb][:, 128 * (g - kb // 2):
                                         128 * (g - kb // 2) + 128]
                else:
                    L = 16 - 2 * g
                    row = 512 + 128 * g

                    def slc(kb):
                        return pt_hi[kb][:, 128 * g:128 * (g + 1)]

                lps = psavp.tile([128, 2], F32, tag="psav",
                                 name=f"l_{sec}_{g}")
                for kb in range(L):
                    nc.tensor.matmul(
                        lps,
                        lhsT=slc(kb),
                        rhs=ones,
                        start=(kb == 0),
                        stop=(kb == L - 1),
                    )
                rec = smlp.tile([128, 1], F32, tag="rec")
                nc.vector.reciprocal(rec, lps[:, 0:1])
                for oh in range(2):
                    avp = psavp.tile([128, 512], F32, tag="psav",
                                     name=f"av_{sec}_{g}_{oh}")
                    for kb in range(L):
                        nc.tensor.matmul(
                            avp,
                            lhsT=slc(kb),
                            rhs=vvs[kb // 8][:, oh, kb % 8, :],
                            start=(kb == 0),
                            stop=(kb == L - 1),
                        )
                    ot = otp.tile([128, 512], F32, tag="ot",
                                  name=f"ot_{sec}_{g}_{oh}")
                    nc.vector.tensor_scalar_mul(ot, avp, rec)
                    # split output stores across two queues: the tail
                    # otherwise serializes on a single queue's drain
                    eng = nc.sync if sec == "hi" else nc.scalar
                    eng.dma_start(
                        out=out[row:row + 128, oh * 512:(oh + 1) * 512],
                        in_=ot,
                    )

            # LOW scores first: they only need the first K gather group.
            # AV: LOW blocks first -- they only read the first V gather
            # group, buying the trailing V gather ~15us of slack.
            emit_scores_lo()
            emit_scores_hi()
            for g in (3, 2, 1):
                emit_av("lo", g)
            for g in range(4):
                emit_av("hi", g)
            # the L=2 chain last: shortest possible kernel tail
            emit_av("lo", 0)
    nc.compile()
    return nc


def _query_blocks(sub):
    return [2 * g + sub for g in range(4)] + [15 - 2 * g - sub for g in range(4)]


def _query_cols(sub):
    return np.concatenate(
        [np.arange(b * 128, (b + 1) * 128) for b in _query_blocks(sub)]
    )


def _masks(sub):
    m = np.zeros((16, 128, 128), np.float32)
    k = np.arange(128)[:, None]
    q = np.arange(128)[None, :]
    for kb in range(16):
        if kb < 8:
            block = 2 * (kb // 2) + sub
        else:
            block = 15 - 2 * ((15 - kb) // 2) - sub
        m[kb] = (kb * 128 + k <= block * 128 + q).astype(np.float32)
    return np.ascontiguousarray(m.transpose(1, 0, 2))  # -> [128, 16, 128]


def _pmaj(a):
    """[dc*128, cols] -> partition-major [128, dc, cols]."""
    d, cols = a.shape
    return np.ascontiguousarray(a.reshape(d // 128, 128, cols).transpose(1, 0, 2))


def kernel(x, Wq, Wk, Wv, _trace=False):
    if "nc" not in _cache:
        _cache["nc"] = build_nc()
    nc = _cache["nc"]

    bf = ml_dtypes.bfloat16
    x = np.asarray(x, dtype=np.float32)
    WqT = _pmaj((np.asarray(Wq, np.float32).T * np.float32(SCALE)).astype(bf))
    WkT = np.asarray(Wk, np.float32).T.astype(bf)
    WvT = np.asarray(Wv, np.float32).T.astype(bf)

    in_maps = []
    for c in range(8):
        b, sub = c // 2, c % 2
        xT = x[b].T.astype(bf)
        in_maps.append(
            {
                "xT": _pmaj(xT),
                "xTq": _pmaj(np.ascontiguousarray(xT[:, _query_cols(sub)])),
                "WqT": WqT,
                "WkTh": _pmaj(WkT[:, sub * 512:(sub + 1) * 512]),
                "WvTh": _pmaj(WvT[:, sub * 512:(sub + 1) * 512]),
                "masks": _masks(sub).astype(bf),
            }
        )

    res = run_bass_kernel_spmd(
        nc, in_maps, core_ids=list(range(8)), trace=_trace
    )
    full = np.empty((B, S, D), np.float32)
    for c in range(8):
        b, sub = c // 2, c % 2
        full[b, _query_cols(sub)] = res.results[c]["out"]
    if _trace:
        _cache["last_result"] = res
    return full
